# revision 31
# baseline (speedup 1.0000x reference)
"""Trainium2 Bass kernel for nn_Net_34729105555716.

Model: embedding lookup [30000,100] -> input projection (w_ih) -> 200-step
tanh RNN (hidden 300) -> relu MLP (300->256->3) over batch 4096.

Strategy (data-parallel over batch, 512 rows per core, 8 cores), v3:
  - fp16 embedding table in SBUF in dma_gather tokens_per_rank=128 layout;
    table columns 100/101 are a constant 1.0 so the fused matmul can carry
    the RNN bias (b_ih+b_hh) as two e4m3 lhsT rows (hi + residual). One
    transposed SWDGE gather per step pulls 512 tokens into
    [emb(partitions), batch]; a DVE tensor_copy converts them to fp8 into
    slot 0 of the per-stream state tile.
  - The per-core batch of 512 is split into two interleaved 256-column
    streams so each stream's tanh overlaps the other stream's work.
  - Steps 0..191 do the WHOLE per-step GEMM as 6 fp8(e4m3) DoubleRow
    matmuls per stream (0.5 cycles/row): 3 "fused" (slot pair =
    [xe+ones | hidden rows 256..299]) + 3 "recurrent" (slot pair =
    [h rows 0..127 | h rows 128..255]). The per-stream state tile t8
    [128, 4(slots), 256] is exactly the rhs layout: tanh writes slots 1..3
    directly, the DVE convert writes slot 0. Keeping the input projection
    inside the DoubleRow group (instead of separate start=True matmuls)
    keeps the serial tanh->matmul->tanh chain at ~1.3us/stream, under the
    ScalarE bound, without needing double-buffered PSUM banks.
  - Weights are pre-scaled by S=64 (fp8 subnormal hygiene); tanh applies
    scale=1/S.
  - Steps 192..199 run in fp16 (h kept fp16): the tanh recurrence is
    contractive, so fp8 quantization error injected before the tail decays
    to the fp16 noise floor (verified numerically: ~6e-4 absmax-rel).
  - Each stream's tanh is ONE ScalarE instruction over its 3 PSUM banks
    ([128, 3, 256] in -> [128, 3, 256] out), bias-free thanks to the
    ones-row trick.
  - MLP head in fp16 from the final fp16 h tiles; fc1 relu fused on
    ScalarE, fc2 bias on VectorE.
Host side marshals inputs (dtype cast/scale, transpose, fp8 packing,
index layout) and transposes the per-core [3, 512] outputs to [4096, 3].
"""

import sys

if "/opt/trn_rl_repo" not in sys.path:
    sys.path.insert(0, "/opt/trn_rl_repo")

import numpy as np

SEQ = 200
BATCH = 4096
VOCAB = 30000
EMB = 100
HID = 300
HIDP = 304  # fp8 DoubleRow lhsT slot stride must be a multiple of 16
FC1 = 256
N_CORES = 8
BPC = BATCH // N_CORES  # batch per core
NSTR = 2  # interleaved streams per core
CPS = BPC // NSTR  # columns per stream
N_RANKS = (VOCAB + 127) // 128  # 235
KT = [(0, 128), (128, 128), (256, 44)]  # hidden-dim tiles (fp16 path)
# M-tile -> psum bank (and tanh slot) order: slot0 = rows 256..299,
# slot1 = rows 0..127, slot2 = rows 128..255
BANKS = [(256, 44), (0, 128), (128, 128)]
TAIL = 6  # trailing steps computed in fp16
WARM = 24  # leading steps fed host-gathered xe (hides the table load)
WS = 64.0  # weight pre-scale for fp8

_cached = {}


def _split_multiwait(nc, mybir):
    """walrus in this container rejects >1 embedded sync wait per
    instruction (>2 for EventSemaphore); split extras onto NoOp carriers."""
    n = 0
    for f in nc.m.functions:
        for blk in f.blocks:
            if not any(
                i.sync_info is not None and len(i.sync_info.on_wait) > 1
                for i in blk.instructions
            ):
                continue
            out = []
            for inst in blk.instructions:
                si = inst.sync_info
                cap = 2 if isinstance(inst, mybir.InstEventSemaphore) else 1
                if si is not None and len(si.on_wait) > cap:
                    waits = list(si.on_wait)
                    for w in waits[:-cap]:
                        n += 1
                        carrier = mybir.InstNoOp(
                            name=f"I-waitsplit-{n}", ins=[], outs=[]
                        )
                        carrier.engine = inst.engine
                        carrier.sync_info = mybir.SyncInfo(
                            on_wait=[w], on_update=[]
                        )
                        out.append(carrier)
                    si.on_wait = waits[-cap:]
                out.append(inst)
            blk.instructions = out
    return n


def _build(seq=SEQ):
    import concourse.bass as bass
    import concourse.mybir as mybir
    import concourse.tile as tile
    from concourse import library_config
    from concourse.tile import add_dep_helper

    dt = mybir.dt
    f16, f32, i16, f8 = dt.float16, dt.float32, dt.int16, dt.float8e4
    Tanh = mybir.ActivationFunctionType.Tanh
    Relu = mybir.ActivationFunctionType.Relu
    DR = mybir.MatmulPerfMode.DoubleRow

    nc = bass.Bass(
        "TRN2", target_bir_lowering=False, debug=False, num_devices=N_CORES
    )
    x_idx = nc.dram_tensor(
        "x_idx", [128, seq * BPC // 16], i16, kind="ExternalInput"
    )
    emb_sb = nc.dram_tensor(
        "emb_sb", [128, N_RANKS * 128], f16, kind="ExternalInput"
    )
    # fp16 weights (input projection carries bias row; everything the PSUM
    # accumulates is pre-scaled by WS, undone by tanh's scale=1/WS)
    wih_t = nc.dram_tensor("wih_t", [EMB + 1, HID], f16, kind="ExternalInput")
    whh_t = nc.dram_tensor("whh_t", [HID, HID], f16, kind="ExternalInput")
    # fp8 DoubleRow packs: "fused" = [wih+bias rows | whh rows 256..299],
    # "recurrent" = [whh rows 0..127 | whh rows 128..255]
    wfu_t = nc.dram_tensor("wfu_t", [128, 2 * HIDP], f8, kind="ExternalInput")
    wdr_t = nc.dram_tensor("wdr_t", [128, 2 * HIDP], f8, kind="ExternalInput")
    fc1_t = nc.dram_tensor("fc1_t", [HID, FC1], f16, kind="ExternalInput")
    fc2_t = nc.dram_tensor("fc2_t", [FC1, 3], f16, kind="ExternalInput")
    fc1b_sb = nc.dram_tensor("fc1b_sb", [128, 2], f32, kind="ExternalInput")
    fc2b_sb = nc.dram_tensor("fc2b_sb", [3, 1], f32, kind="ExternalInput")
    # host-gathered embeddings for the first WARM steps: the RNN starts on
    # these (~1us in) while the 7.7MB table streams into SBUF behind them
    xe_warm = nc.dram_tensor(
        "xe_warm", [128, WARM * BPC], f16, kind="ExternalInput"
    )
    out = nc.dram_tensor("out", [3, BPC], f32, kind="ExternalOutput")

    with tile.TileContext(nc) as tc:
        with (
            tc.tile_pool(name="const", bufs=1) as cpool,
            tc.tile_pool(name="gather", bufs=4) as gpool,
            tc.tile_pool(name="psum", bufs=1, space="PSUM") as ppool,
        ):
            lib_inst = nc.gpsimd.load_library(library_config.mlp)

            # DMA need-order, split across the two HWDGE rings: the fp8
            # weights ride the otherwise-idle Activation ring while the SP
            # ring streams warm-start xe granules (step t needs granule
            # t//4), then idx + table (first gather, step WARM), then the
            # fp16-path weights (first needed at step seq-TAIL).
            wfu = cpool.tile([128, 2, HIDP], f8, tag="wfu")
            nc.scalar.dma_start(wfu[:], wfu_t.ap())
            wdr = cpool.tile([128, 2, HIDP], f8, tag="wdr")
            nc.scalar.dma_start(wdr[:], wdr_t.ap())
            warm = cpool.tile([128, WARM, BPC], f16, tag="warm")
            warm_granules = [(0, 2), (2, 2)] + [
                (g, 4) for g in range(4, WARM, 4)
            ]
            for g0, gn in warm_granules:
                nc.sync.dma_start(
                    warm[:, g0 : g0 + gn, :],
                    xe_warm.ap()[:, g0 * BPC : (g0 + gn) * BPC],
                )
            idx = cpool.tile([128, seq * BPC // 16], i16, tag="idx")
            nc.sync.dma_start(idx[:], x_idx.ap())
            tbl = cpool.tile([128, N_RANKS * 128], f16, tag="tbl")
            nc.sync.dma_start(tbl[:], emb_sb.ap())

            wih = cpool.tile([EMB + 1, HID], f16, tag="wih")
            nc.sync.dma_start(wih[:], wih_t.ap())
            whh = []
            for o, sz in KT:
                w = cpool.tile([sz, HID], f16, tag=f"whh{o}")
                nc.sync.dma_start(w[:], whh_t.ap()[o : o + sz, :])
                whh.append(w)
            fc1 = []
            for o, sz in KT:
                w = cpool.tile([sz, FC1], f16, tag=f"fc1{o}")
                nc.sync.dma_start(w[:], fc1_t.ap()[o : o + sz, :])
                fc1.append(w)
            fc2 = []
            for o in (0, 128):
                w = cpool.tile([128, 3], f16, tag=f"fc2{o}")
                nc.sync.dma_start(w[:], fc2_t.ap()[o : o + 128, :])
                fc2.append(w)
            fc1b_t = cpool.tile([128, 2], f32, tag="fc1b")
            nc.sync.dma_start(fc1b_t[:], fc1b_sb.ap())
            fc2b_t = cpool.tile([3, 1], f32, tag="fc2b")
            nc.sync.dma_start(fc2b_t[:], fc2b_sb.ap())

            reg_n = nc.gpsimd.to_reg(BPC)

            # per-stream state: [xe8 | h rows 256..299 | h 0..127 | h 128..255]
            # (exactly the two DoubleRow rhs slot pairs), fp16 h for the tail,
            # relu output, staging for the final store
            t8 = [
                cpool.tile([128, 4, CPS], f8, tag=f"t8_{s}", name=f"t8_{s}")
                for s in range(NSTR)
            ]
            h16 = [
                cpool.tile([128, 3, CPS], f16, tag=f"h16_{s}", name=f"h16_{s}")
                for s in range(NSTR)
            ]
            h1 = [
                cpool.tile([128, 2, CPS], f16, tag=f"h1_{s}", name=f"h1_{s}")
                for s in range(NSTR)
            ]
            osb = [
                cpool.tile([3, CPS], f32, tag=f"osb_{s}", name=f"osb_{s}") for s in range(NSTR)
            ]
            ps = [
                ppool.tile([128, 3, 512], f32, tag=f"ps_{s}", name=f"ps_{s}")
                for s in range(NSTR)
            ]
            # zero init: t8 h-slots only (slot 0 is overwritten by the first
            # convert) on DVE; psum bank0 (partitions 44..127: never
            # matmul-written but tanh-read) via the startup-idle ScalarE
            for s in range(NSTR):
                nc.vector.memset(t8[s][:, 1:4, :], 0)
                nc.scalar.memzero(ps[s][:, 0, :])

            for t in range(seq):
                if t < WARM:
                    xgv = lambda p, c, _t=t: warm[p, _t, c]
                else:
                    xg = gpool.tile([128, 1, BPC], f16, tag="xe")
                    gi = nc.gpsimd.dma_gather(
                        xg[:],
                        tbl[:],
                        idx[:, t * (BPC // 16) : (t + 1) * (BPC // 16)],
                        BPC,
                        reg_n,
                        128,
                        transpose=True,
                        sbuf_tokens_per_rank=128,
                        sbuf_free_dim_per_rank=256,
                    )
                    add_dep_helper(
                        gi.ins, lib_inst.ins, sync=False, reason="lib first"
                    )
                    xgv = lambda p, c, _g=xg: _g[p, 0, c]
                fp8_mm = t < seq - TAIL  # recurrent matmul precision
                fp8_out = t < seq - TAIL - 1  # tanh output precision
                if fp8_mm:
                    # fp8 xe into slot 0 of each stream's state tile (the
                    # WAR on last step's fused matmul orders this; in steady
                    # state it runs ~1 step ahead on the otherwise-idle DVE)
                    for s in range(NSTR):
                        cols = slice(s * CPS, (s + 1) * CPS)
                        nc.vector.tensor_copy(
                            t8[s][:, 0, :], xgv(slice(0, 128), cols)
                        )
                for s in range(NSTR):
                    cols = slice(s * CPS, (s + 1) * CPS)
                    if fp8_mm:
                        for j, (mo, ms) in enumerate(BANKS):
                            nc.tensor.matmul(
                                ps[s][0:ms, j, cols],
                                wfu[:, :, mo : mo + ms],
                                t8[s][:, 0:2, :],
                                start=True,
                                stop=False,
                                perf_mode=DR,
                            )
                        for j, (mo, ms) in enumerate(BANKS):
                            nc.tensor.matmul(
                                ps[s][0:ms, j, cols],
                                wdr[:, :, mo : mo + ms],
                                t8[s][:, 2:4, :],
                                start=False,
                                stop=True,
                                perf_mode=DR,
                            )
                    else:
                        for j, (mo, ms) in enumerate(BANKS):
                            nc.tensor.matmul(
                                ps[s][0:ms, j, cols],
                                wih[:, mo : mo + ms],
                                xgv(slice(0, EMB + 1), cols),
                                start=True,
                                stop=False,
                            )
                        for ki, (ko, ks) in enumerate(KT):
                            rhs = (
                                h16[s][0:44, 0, :]
                                if ks == 44
                                else h16[s][:, 1 + ki, :]
                            )
                            last = ki == len(KT) - 1
                            for j, (mo, ms) in enumerate(BANKS):
                                nc.tensor.matmul(
                                    ps[s][0:ms, j, cols],
                                    whh[ki][:, mo : mo + ms],
                                    rhs,
                                    start=False,
                                    stop=last,
                                )
                    dst = t8[s][:, 1:4, :] if fp8_out else h16[s][:, 0:3, :]
                    nc.scalar.activation(
                        dst,
                        ps[s][:, 0:3, cols],
                        Tanh,
                        scale=1.0 / WS,
                    )

            # MLP head (fp16)
            for s in range(NSTR):
                for mi in range(2):
                    for ki, (ko, ks) in enumerate(KT):
                        rhs = (
                            h16[s][0:44, 0, :]
                            if ks == 44
                            else h16[s][:, 1 + ki, :]
                        )
                        nc.tensor.matmul(
                            ps[s][:, mi, 0:CPS],
                            fc1[ki][:, mi * 128 : (mi + 1) * 128],
                            rhs,
                            start=(ki == 0),
                            stop=(ki == len(KT) - 1),
                        )
                    nc.scalar.activation(
                        h1[s][:, mi, :],
                        ps[s][:, mi, 0:CPS],
                        Relu,
                        bias=fc1b_t[:, mi : mi + 1],
                    )
                nc.tensor.matmul(
                    ps[s][0:3, 2, 0:CPS],
                    fc2[0][:, :],
                    h1[s][:, 0, :],
                    start=True,
                    stop=False,
                )
                nc.tensor.matmul(
                    ps[s][0:3, 2, 0:CPS],
                    fc2[1][:, :],
                    h1[s][:, 1, :],
                    start=False,
                    stop=True,
                )
                nc.vector.tensor_scalar_add(
                    osb[s][:], ps[s][0:3, 2, 0:CPS], fc2b_t[:, 0:1]
                )
                nc.sync.dma_start(
                    out.ap()[:, s * CPS : (s + 1) * CPS], osb[s][:]
                )

    # Populate .instr bytes for InstISA subclasses (library reload etc.) —
    # Bacc.compile does this; raw Bass+Tile must do it explicitly or walrus
    # fails with "ISA wrong length".
    mybir.codegen_inst_isa_subclasses(nc)
    _split_multiwait(nc, mybir)
    return nc


def _prep_inputs(x, emb, w_ih, w_hh, b_ih, b_hh, fc1_w, fc1_b, fc2_w, fc2_b,
                 seq=SEQ):
    """Marshal the model inputs into per-core DRAM input maps."""
    import ml_dtypes

    E4M3 = ml_dtypes.float8_e4m3

    x = np.asarray(x)
    assert x.shape == (seq, BATCH), x.shape

    # Embedding table in SBUF-gather layout: partition = token % 128,
    # rank = token // 128, 128 fp16 elems (256B) per row. Columns 100/101
    # are a constant 1.0: ones-rows that carry the bias (as an e4m3
    # hi + residual pair) through the fused matmul.
    emb_pad = np.zeros((N_RANKS * 128, 128), np.float16)
    emb_pad[:VOCAB, :EMB] = np.asarray(emb, np.float16)
    emb_pad[:, EMB] = 1.0
    emb_pad[:, EMB + 1] = 1.0
    emb_sb = np.ascontiguousarray(
        emb_pad.reshape(N_RANKS, 128, 128).transpose(1, 0, 2).reshape(128, -1)
    )

    bias = (np.asarray(b_ih, np.float64) + np.asarray(b_hh, np.float64))
    wih_aug = np.concatenate(
        [np.asarray(w_ih, np.float64).T, bias[None, :]], axis=0
    )  # [101, 300]
    wih_t = np.ascontiguousarray((wih_aug * WS).astype(np.float16))

    whh_sc = np.asarray(w_hh, np.float64).T * WS  # [in, out] scaled
    whh_t = np.ascontiguousarray(whh_sc.astype(np.float16))
    # fp8 DoubleRow packs.
    # fused pack slot 0: scaled w_ih rows, bias-hi (row 100), bias-residual
    # (row 101); slot 1: whh rows 256..299.
    bias_hi = (bias * WS).astype(E4M3)
    bias_res = (bias * WS - bias_hi.astype(np.float64)).astype(E4M3)
    wfu = np.zeros((128, 2, HIDP), E4M3)
    wfu[0:EMB, 0, :HID] = (np.asarray(w_ih, np.float64).T * WS).astype(E4M3)
    wfu[EMB, 0, :HID] = bias_hi
    wfu[EMB + 1, 0, :HID] = bias_res
    wfu[0:44, 1, :HID] = whh_sc[256:300, :].astype(E4M3)
    wfu_t = np.ascontiguousarray(wfu).reshape(128, 2 * HIDP)
    # recurrent pack: wdr[k, s, m] = whh_sc[s*128 + k, m]
    wdr = np.zeros((128, 2, HIDP), E4M3)
    wdr[:, :, :HID] = np.ascontiguousarray(
        whh_sc[0:256, :].reshape(2, 128, HID).transpose(1, 0, 2)
    ).astype(E4M3)
    wdr_t = wdr.reshape(128, 2 * HIDP)

    fc1_t = np.ascontiguousarray(np.asarray(fc1_w, np.float16).T)  # [hid, 256]
    fc2_t = np.ascontiguousarray(np.asarray(fc2_w, np.float16).T)  # [256, 3]
    fc1b_sb = np.ascontiguousarray(
        np.asarray(fc1_b, np.float32).reshape(2, 128).T
    )
    fc2b_sb = np.asarray(fc2_b, np.float32).reshape(3, 1)

    shared = {
        "emb_sb": emb_sb,
        "wih_t": wih_t,
        "whh_t": whh_t,
        "wfu_t": wfu_t,
        "wdr_t": wdr_t,
        "fc1_t": fc1_t,
        "fc2_t": fc2_t,
        "fc1b_sb": fc1b_sb,
        "fc2b_sb": fc2b_sb,
    }
    in_maps = []
    for c in range(N_CORES):
        xc = x[:, c * BPC : (c + 1) * BPC]  # [seq, 512]
        flat = np.ascontiguousarray(xc).reshape(-1).astype(np.int16)
        block = np.ascontiguousarray(flat.reshape(-1, 16).T)  # [16, seq*BPC/16]
        x_idx = np.ascontiguousarray(np.tile(block, (8, 1)))  # [128, ...]
        # host-gathered first WARM steps: same layout a transposed gather
        # would produce, [128(row elems), WARM*BPC(tokens)]
        xw = emb_pad[xc[:WARM].reshape(-1)]  # [WARM*BPC, 128]
        xe_warm = np.ascontiguousarray(xw.T)
        in_maps.append({"x_idx": x_idx, "xe_warm": xe_warm, **shared})
    return in_maps


def _get_nc():
    if "nc" not in _cached:
        _cached["nc"] = _build()
    return _cached["nc"]


def kernel(x, emb, w_ih, w_hh, b_ih, b_hh, fc1_w, fc1_b, fc2_w, fc2_b):
    from concourse.bass_utils import run_bass_kernel_spmd

    nc = _get_nc()
    in_maps = _prep_inputs(
        x, emb, w_ih, w_hh, b_ih, b_hh, fc1_w, fc1_b, fc2_w, fc2_b
    )
    res = run_bass_kernel_spmd(nc, in_maps, core_ids=list(range(N_CORES)))
    # per-core out is [3, 512]; assemble full [4096, 3]
    full = np.concatenate([r["out"].T for r in res.results], axis=0)
    return full.astype(np.float32)


# revision 35
# speedup vs baseline: 1.0028x; 1.0028x over previous
"""Trainium2 Bass kernel for nn_Net_34729105555716.

Model: embedding lookup [30000,100] -> input projection (w_ih) -> 200-step
tanh RNN (hidden 300) -> relu MLP (300->256->3) over batch 4096.

Strategy (data-parallel over batch, 512 rows per core, 8 cores), v3:
  - fp16 embedding table in SBUF in dma_gather tokens_per_rank=128 layout;
    table columns 100/101 are a constant 1.0 so the fused matmul can carry
    the RNN bias (b_ih+b_hh) as two e4m3 lhsT rows (hi + residual). One
    transposed SWDGE gather per step pulls 512 tokens into
    [emb(partitions), batch]; a DVE tensor_copy converts them to fp8 into
    slot 0 of the per-stream state tile.
  - The per-core batch of 512 is split into two interleaved 256-column
    streams so each stream's tanh overlaps the other stream's work.
  - Steps 0..191 do the WHOLE per-step GEMM as 6 fp8(e4m3) DoubleRow
    matmuls per stream (0.5 cycles/row): 3 "fused" (slot pair =
    [xe+ones | hidden rows 256..299]) + 3 "recurrent" (slot pair =
    [h rows 0..127 | h rows 128..255]). The per-stream state tile t8
    [128, 4(slots), 256] is exactly the rhs layout: tanh writes slots 1..3
    directly, the DVE convert writes slot 0. Keeping the input projection
    inside the DoubleRow group (instead of separate start=True matmuls)
    keeps the serial tanh->matmul->tanh chain at ~1.3us/stream, under the
    ScalarE bound, without needing double-buffered PSUM banks.
  - Weights are pre-scaled by S=64 (fp8 subnormal hygiene); tanh applies
    scale=1/S.
  - Steps 192..199 run in fp16 (h kept fp16): the tanh recurrence is
    contractive, so fp8 quantization error injected before the tail decays
    to the fp16 noise floor (verified numerically: ~6e-4 absmax-rel).
  - Each stream's tanh is ONE ScalarE instruction over its 3 PSUM banks
    ([128, 3, 256] in -> [128, 3, 256] out), bias-free thanks to the
    ones-row trick.
  - MLP head in fp16 from the final fp16 h tiles; fc1 relu fused on
    ScalarE, fc2 bias on VectorE.
Host side marshals inputs (dtype cast/scale, transpose, fp8 packing,
index layout) and transposes the per-core [3, 512] outputs to [4096, 3].
"""

import sys

if "/opt/trn_rl_repo" not in sys.path:
    sys.path.insert(0, "/opt/trn_rl_repo")

import numpy as np

SEQ = 200
BATCH = 4096
VOCAB = 30000
EMB = 100
HID = 300
HIDP = 304  # fp8 DoubleRow lhsT slot stride must be a multiple of 16
FC1 = 256
N_CORES = 8
BPC = BATCH // N_CORES  # batch per core
NSTR = 2  # interleaved streams per core
CPS = BPC // NSTR  # columns per stream
N_RANKS = (VOCAB + 127) // 128  # 235
KT = [(0, 128), (128, 128), (256, 44)]  # hidden-dim tiles (fp16 path)
# M-tile -> psum bank (and tanh slot) order: slot0 = rows 256..299,
# slot1 = rows 0..127, slot2 = rows 128..255
BANKS = [(256, 44), (0, 128), (128, 128)]
TAIL = 5  # trailing steps computed in fp16
WARM = 24  # leading steps fed host-gathered xe (hides the table load)
WS = 64.0  # weight pre-scale for fp8

_cached = {}


def _split_multiwait(nc, mybir):
    """walrus in this container rejects >1 embedded sync wait per
    instruction (>2 for EventSemaphore); split extras onto NoOp carriers."""
    n = 0
    for f in nc.m.functions:
        for blk in f.blocks:
            if not any(
                i.sync_info is not None and len(i.sync_info.on_wait) > 1
                for i in blk.instructions
            ):
                continue
            out = []
            for inst in blk.instructions:
                si = inst.sync_info
                cap = 2 if isinstance(inst, mybir.InstEventSemaphore) else 1
                if si is not None and len(si.on_wait) > cap:
                    waits = list(si.on_wait)
                    for w in waits[:-cap]:
                        n += 1
                        carrier = mybir.InstNoOp(
                            name=f"I-waitsplit-{n}", ins=[], outs=[]
                        )
                        carrier.engine = inst.engine
                        carrier.sync_info = mybir.SyncInfo(
                            on_wait=[w], on_update=[]
                        )
                        out.append(carrier)
                    si.on_wait = waits[-cap:]
                out.append(inst)
            blk.instructions = out
    return n


def _build(seq=SEQ):
    import concourse.bass as bass
    import concourse.mybir as mybir
    import concourse.tile as tile
    from concourse import library_config
    from concourse.tile import add_dep_helper

    dt = mybir.dt
    f16, f32, i16, f8 = dt.float16, dt.float32, dt.int16, dt.float8e4
    Tanh = mybir.ActivationFunctionType.Tanh
    Relu = mybir.ActivationFunctionType.Relu
    DR = mybir.MatmulPerfMode.DoubleRow

    nc = bass.Bass(
        "TRN2", target_bir_lowering=False, debug=False, num_devices=N_CORES
    )
    x_idx = nc.dram_tensor(
        "x_idx", [128, seq * BPC // 16], i16, kind="ExternalInput"
    )
    emb_sb = nc.dram_tensor(
        "emb_sb", [128, N_RANKS * 128], f16, kind="ExternalInput"
    )
    # fp16 weights (input projection carries bias row; everything the PSUM
    # accumulates is pre-scaled by WS, undone by tanh's scale=1/WS)
    wih_t = nc.dram_tensor("wih_t", [EMB + 1, HID], f16, kind="ExternalInput")
    whh_t = nc.dram_tensor("whh_t", [HID, HID], f16, kind="ExternalInput")
    # fp8 DoubleRow packs: "fused" = [wih+bias rows | whh rows 256..299],
    # "recurrent" = [whh rows 0..127 | whh rows 128..255]
    wfu_t = nc.dram_tensor("wfu_t", [128, 2 * HIDP], f8, kind="ExternalInput")
    wdr_t = nc.dram_tensor("wdr_t", [128, 2 * HIDP], f8, kind="ExternalInput")
    fc1_t = nc.dram_tensor("fc1_t", [HID, FC1], f16, kind="ExternalInput")
    fc2_t = nc.dram_tensor("fc2_t", [FC1, 3], f16, kind="ExternalInput")
    fc1b_sb = nc.dram_tensor("fc1b_sb", [128, 2], f32, kind="ExternalInput")
    fc2b_sb = nc.dram_tensor("fc2b_sb", [3, 1], f32, kind="ExternalInput")
    # host-gathered embeddings for the first WARM steps: the RNN starts on
    # these (~1us in) while the 7.7MB table streams into SBUF behind them
    xe_warm = nc.dram_tensor(
        "xe_warm", [128, WARM * BPC], f16, kind="ExternalInput"
    )
    out = nc.dram_tensor("out", [3, BPC], f32, kind="ExternalOutput")

    with tile.TileContext(nc) as tc:
        with (
            tc.tile_pool(name="const", bufs=1) as cpool,
            tc.tile_pool(name="gather", bufs=4) as gpool,
            tc.tile_pool(name="psum", bufs=1, space="PSUM") as ppool,
        ):
            # DMA need-order, split across the two HWDGE rings: the fp8
            # weights ride the otherwise-idle Activation ring while the SP
            # ring streams warm-start xe granules (step t needs its granule),
            # then idx + table (first gather, step WARM), then the
            # fp16-path weights (first needed at step seq-TAIL). The gpsimd
            # library load (only needed by the first gather) is emitted
            # after the warm granules so it doesn't head-of-line block them.
            wfu = cpool.tile([128, 2, HIDP], f8, tag="wfu")
            nc.scalar.dma_start(wfu[:], wfu_t.ap())
            wdr = cpool.tile([128, 2, HIDP], f8, tag="wdr")
            nc.scalar.dma_start(wdr[:], wdr_t.ap())
            warm = cpool.tile([128, WARM, BPC], f16, tag="warm")
            warm_granules = [(0, 2), (2, 2)] + [
                (g, 4) for g in range(4, WARM, 4)
            ]
            for g0, gn in warm_granules:
                nc.sync.dma_start(
                    warm[:, g0 : g0 + gn, :],
                    xe_warm.ap()[:, g0 * BPC : (g0 + gn) * BPC],
                )
            lib_inst = nc.gpsimd.load_library(library_config.mlp)
            idx = cpool.tile([128, seq * BPC // 16], i16, tag="idx")
            nc.sync.dma_start(idx[:], x_idx.ap())
            tbl = cpool.tile([128, N_RANKS * 128], f16, tag="tbl")
            nc.sync.dma_start(tbl[:], emb_sb.ap())

            wih = cpool.tile([EMB + 1, HID], f16, tag="wih")
            nc.sync.dma_start(wih[:], wih_t.ap())
            whh = []
            for o, sz in KT:
                w = cpool.tile([sz, HID], f16, tag=f"whh{o}")
                nc.sync.dma_start(w[:], whh_t.ap()[o : o + sz, :])
                whh.append(w)
            fc1 = []
            for o, sz in KT:
                w = cpool.tile([sz, FC1], f16, tag=f"fc1{o}")
                nc.sync.dma_start(w[:], fc1_t.ap()[o : o + sz, :])
                fc1.append(w)
            fc2 = []
            for o in (0, 128):
                w = cpool.tile([128, 3], f16, tag=f"fc2{o}")
                nc.sync.dma_start(w[:], fc2_t.ap()[o : o + 128, :])
                fc2.append(w)
            fc1b_t = cpool.tile([128, 2], f32, tag="fc1b")
            nc.sync.dma_start(fc1b_t[:], fc1b_sb.ap())
            fc2b_t = cpool.tile([3, 1], f32, tag="fc2b")
            nc.sync.dma_start(fc2b_t[:], fc2b_sb.ap())

            reg_n = nc.gpsimd.to_reg(BPC)

            # per-stream state: [xe8 | h rows 256..299 | h 0..127 | h 128..255]
            # (exactly the two DoubleRow rhs slot pairs), fp16 h for the tail,
            # relu output, staging for the final store
            t8 = [
                cpool.tile([128, 4, CPS], f8, tag=f"t8_{s}", name=f"t8_{s}")
                for s in range(NSTR)
            ]
            h16 = [
                cpool.tile([128, 3, CPS], f16, tag=f"h16_{s}", name=f"h16_{s}")
                for s in range(NSTR)
            ]
            h1 = [
                cpool.tile([128, 2, CPS], f16, tag=f"h1_{s}", name=f"h1_{s}")
                for s in range(NSTR)
            ]
            osb = [
                cpool.tile([3, CPS], f32, tag=f"osb_{s}", name=f"osb_{s}") for s in range(NSTR)
            ]
            ps = [
                ppool.tile([128, 3, 512], f32, tag=f"ps_{s}", name=f"ps_{s}")
                for s in range(NSTR)
            ]
            # zero init: t8 h-slots only (slot 0 is overwritten by the first
            # convert) on DVE; psum bank0 (partitions 44..127: never
            # matmul-written but tanh-read) via the startup-idle ScalarE
            for s in range(NSTR):
                nc.vector.memset(t8[s][:, 1:4, :], 0)
                nc.scalar.memzero(ps[s][:, 0, :])

            for t in range(seq):
                if t < WARM:
                    xgv = lambda p, c, _t=t: warm[p, _t, c]
                else:
                    xg = gpool.tile([128, 1, BPC], f16, tag="xe")
                    gi = nc.gpsimd.dma_gather(
                        xg[:],
                        tbl[:],
                        idx[:, t * (BPC // 16) : (t + 1) * (BPC // 16)],
                        BPC,
                        reg_n,
                        128,
                        transpose=True,
                        sbuf_tokens_per_rank=128,
                        sbuf_free_dim_per_rank=256,
                    )
                    add_dep_helper(
                        gi.ins, lib_inst.ins, sync=False, reason="lib first"
                    )
                    xgv = lambda p, c, _g=xg: _g[p, 0, c]
                fp8_mm = t < seq - TAIL  # recurrent matmul precision
                fp8_out = t < seq - TAIL - 1  # tanh output precision
                if fp8_mm:
                    # fp8 xe into slot 0 of each stream's state tile (the
                    # WAR on last step's fused matmul orders this; in steady
                    # state it runs ~1 step ahead on the otherwise-idle DVE)
                    for s in range(NSTR):
                        cols = slice(s * CPS, (s + 1) * CPS)
                        nc.vector.tensor_copy(
                            t8[s][:, 0, :], xgv(slice(0, 128), cols)
                        )
                for s in range(NSTR):
                    cols = slice(s * CPS, (s + 1) * CPS)
                    if fp8_mm:
                        for j, (mo, ms) in enumerate(BANKS):
                            nc.tensor.matmul(
                                ps[s][0:ms, j, cols],
                                wfu[:, :, mo : mo + ms],
                                t8[s][:, 0:2, :],
                                start=True,
                                stop=False,
                                perf_mode=DR,
                            )
                        for j, (mo, ms) in enumerate(BANKS):
                            nc.tensor.matmul(
                                ps[s][0:ms, j, cols],
                                wdr[:, :, mo : mo + ms],
                                t8[s][:, 2:4, :],
                                start=False,
                                stop=True,
                                perf_mode=DR,
                            )
                    else:
                        for j, (mo, ms) in enumerate(BANKS):
                            nc.tensor.matmul(
                                ps[s][0:ms, j, cols],
                                wih[:, mo : mo + ms],
                                xgv(slice(0, EMB + 1), cols),
                                start=True,
                                stop=False,
                            )
                        for ki, (ko, ks) in enumerate(KT):
                            rhs = (
                                h16[s][0:44, 0, :]
                                if ks == 44
                                else h16[s][:, 1 + ki, :]
                            )
                            last = ki == len(KT) - 1
                            for j, (mo, ms) in enumerate(BANKS):
                                nc.tensor.matmul(
                                    ps[s][0:ms, j, cols],
                                    whh[ki][:, mo : mo + ms],
                                    rhs,
                                    start=False,
                                    stop=last,
                                )
                    dst = t8[s][:, 1:4, :] if fp8_out else h16[s][:, 0:3, :]
                    nc.scalar.activation(
                        dst,
                        ps[s][:, 0:3, cols],
                        Tanh,
                        scale=1.0 / WS,
                    )

            # MLP head (fp16)
            for s in range(NSTR):
                for mi in range(2):
                    for ki, (ko, ks) in enumerate(KT):
                        rhs = (
                            h16[s][0:44, 0, :]
                            if ks == 44
                            else h16[s][:, 1 + ki, :]
                        )
                        nc.tensor.matmul(
                            ps[s][:, mi, 0:CPS],
                            fc1[ki][:, mi * 128 : (mi + 1) * 128],
                            rhs,
                            start=(ki == 0),
                            stop=(ki == len(KT) - 1),
                        )
                    nc.scalar.activation(
                        h1[s][:, mi, :],
                        ps[s][:, mi, 0:CPS],
                        Relu,
                        bias=fc1b_t[:, mi : mi + 1],
                    )
                nc.tensor.matmul(
                    ps[s][0:3, 2, 0:CPS],
                    fc2[0][:, :],
                    h1[s][:, 0, :],
                    start=True,
                    stop=False,
                )
                nc.tensor.matmul(
                    ps[s][0:3, 2, 0:CPS],
                    fc2[1][:, :],
                    h1[s][:, 1, :],
                    start=False,
                    stop=True,
                )
                nc.vector.tensor_scalar_add(
                    osb[s][:], ps[s][0:3, 2, 0:CPS], fc2b_t[:, 0:1]
                )
                # stream 0's store rides the Activation ring so the two
                # final DMAs issue in parallel
                (nc.scalar if s == 0 else nc.sync).dma_start(
                    out.ap()[:, s * CPS : (s + 1) * CPS], osb[s][:]
                )

    # Populate .instr bytes for InstISA subclasses (library reload etc.) —
    # Bacc.compile does this; raw Bass+Tile must do it explicitly or walrus
    # fails with "ISA wrong length".
    mybir.codegen_inst_isa_subclasses(nc)
    _split_multiwait(nc, mybir)
    return nc


def _prep_inputs(x, emb, w_ih, w_hh, b_ih, b_hh, fc1_w, fc1_b, fc2_w, fc2_b,
                 seq=SEQ):
    """Marshal the model inputs into per-core DRAM input maps."""
    import ml_dtypes

    E4M3 = ml_dtypes.float8_e4m3

    x = np.asarray(x)
    assert x.shape == (seq, BATCH), x.shape

    # Embedding table in SBUF-gather layout: partition = token % 128,
    # rank = token // 128, 128 fp16 elems (256B) per row. Columns 100/101
    # are a constant 1.0: ones-rows that carry the bias (as an e4m3
    # hi + residual pair) through the fused matmul.
    emb_pad = np.zeros((N_RANKS * 128, 128), np.float16)
    emb_pad[:VOCAB, :EMB] = np.asarray(emb, np.float16)
    emb_pad[:, EMB] = 1.0
    emb_pad[:, EMB + 1] = 1.0
    emb_sb = np.ascontiguousarray(
        emb_pad.reshape(N_RANKS, 128, 128).transpose(1, 0, 2).reshape(128, -1)
    )

    bias = (np.asarray(b_ih, np.float64) + np.asarray(b_hh, np.float64))
    wih_aug = np.concatenate(
        [np.asarray(w_ih, np.float64).T, bias[None, :]], axis=0
    )  # [101, 300]
    wih_t = np.ascontiguousarray((wih_aug * WS).astype(np.float16))

    whh_sc = np.asarray(w_hh, np.float64).T * WS  # [in, out] scaled
    whh_t = np.ascontiguousarray(whh_sc.astype(np.float16))
    # fp8 DoubleRow packs.
    # fused pack slot 0: scaled w_ih rows, bias-hi (row 100), bias-residual
    # (row 101); slot 1: whh rows 256..299.
    bias_hi = (bias * WS).astype(E4M3)
    bias_res = (bias * WS - bias_hi.astype(np.float64)).astype(E4M3)
    wfu = np.zeros((128, 2, HIDP), E4M3)
    wfu[0:EMB, 0, :HID] = (np.asarray(w_ih, np.float64).T * WS).astype(E4M3)
    wfu[EMB, 0, :HID] = bias_hi
    wfu[EMB + 1, 0, :HID] = bias_res
    wfu[0:44, 1, :HID] = whh_sc[256:300, :].astype(E4M3)
    wfu_t = np.ascontiguousarray(wfu).reshape(128, 2 * HIDP)
    # recurrent pack: wdr[k, s, m] = whh_sc[s*128 + k, m]
    wdr = np.zeros((128, 2, HIDP), E4M3)
    wdr[:, :, :HID] = np.ascontiguousarray(
        whh_sc[0:256, :].reshape(2, 128, HID).transpose(1, 0, 2)
    ).astype(E4M3)
    wdr_t = wdr.reshape(128, 2 * HIDP)

    fc1_t = np.ascontiguousarray(np.asarray(fc1_w, np.float16).T)  # [hid, 256]
    fc2_t = np.ascontiguousarray(np.asarray(fc2_w, np.float16).T)  # [256, 3]
    fc1b_sb = np.ascontiguousarray(
        np.asarray(fc1_b, np.float32).reshape(2, 128).T
    )
    fc2b_sb = np.asarray(fc2_b, np.float32).reshape(3, 1)

    shared = {
        "emb_sb": emb_sb,
        "wih_t": wih_t,
        "whh_t": whh_t,
        "wfu_t": wfu_t,
        "wdr_t": wdr_t,
        "fc1_t": fc1_t,
        "fc2_t": fc2_t,
        "fc1b_sb": fc1b_sb,
        "fc2b_sb": fc2b_sb,
    }
    in_maps = []
    for c in range(N_CORES):
        xc = x[:, c * BPC : (c + 1) * BPC]  # [seq, 512]
        flat = np.ascontiguousarray(xc).reshape(-1).astype(np.int16)
        block = np.ascontiguousarray(flat.reshape(-1, 16).T)  # [16, seq*BPC/16]
        x_idx = np.ascontiguousarray(np.tile(block, (8, 1)))  # [128, ...]
        # host-gathered first WARM steps: same layout a transposed gather
        # would produce, [128(row elems), WARM*BPC(tokens)]
        xw = emb_pad[xc[:WARM].reshape(-1)]  # [WARM*BPC, 128]
        xe_warm = np.ascontiguousarray(xw.T)
        in_maps.append({"x_idx": x_idx, "xe_warm": xe_warm, **shared})
    return in_maps


def _get_nc():
    if "nc" not in _cached:
        _cached["nc"] = _build()
    return _cached["nc"]


def kernel(x, emb, w_ih, w_hh, b_ih, b_hh, fc1_w, fc1_b, fc2_w, fc2_b):
    from concourse.bass_utils import run_bass_kernel_spmd

    nc = _get_nc()
    in_maps = _prep_inputs(
        x, emb, w_ih, w_hh, b_ih, b_hh, fc1_w, fc1_b, fc2_w, fc2_b
    )
    res = run_bass_kernel_spmd(nc, in_maps, core_ids=list(range(N_CORES)))
    # per-core out is [3, 512]; assemble full [4096, 3]
    full = np.concatenate([r["out"].T for r in res.results], axis=0)
    return full.astype(np.float32)
